# revision 2
# baseline (speedup 1.0000x reference)
"""LinkPredictor similarity kernel for 8 Trainium2 NeuronCores.

reference:
    sims = E @ E.T               # [16384, 16384], E = [16384, 512] fp32
    m, M = sims.min(), sims.max()
    sims = (sims - m) / (M - m + 1e-7)
    out  = sims[row_idx, col_idx]     # block-diag strict-upper-tri gather

Key structure exploited:
 - By Cauchy-Schwarz sims[i,j] <= max_k ||e_k||^2 = max diag entry, so the
   global max M is computed exactly on the host from row norms.
 - Only the global MIN needs a full sweep of the matrix. The sweep runs in
   fp8e4m3 with DoubleRow matmuls (2 fp8 MACs/PE-cell/cycle) over the
   upper-triangle slab blocks; per-tile minima are shipped back and the few
   candidate tiles within a conservative fp8-error margin are recomputed
   exactly on the host (tiny: a handful of 128x1024 dot blocks).
 - The 128 gathered diagonal graph blocks live inside the two diagonal
   slab-blocks per core; those are computed in bf16 (error ~1e-4 after
   normalization) and written out directly from the same sweep.

Distribution: N=16384 rows = 16 slabs of 1024. Core c owns row slabs
{c, 15-c}; the 136 upper-triangle slab-pair blocks are covered exactly once
across cores with a fixed shape: per core 2 diagonal blocks (bf16) + 15
off-diagonal blocks (fp8). Per-tile min-reductions are split between DVE
(direct fp32 TensorReduce from PSUM) and ACT-copy->DVE bf16 TensorTensor
running-min (2x DVE mode), balanced by a static model.
"""

import numpy as np
import ml_dtypes

N_GRAPHS = 128
G = 128
D = 512
N = N_GRAPHS * G          # 16384
EPS = 1e-7
NCORES = 8
NSLAB = 16                # 1024-row slabs
SLAB = 1024
KC = D // 128             # 4 contraction chunks of 128
NF8 = 15                  # fp8 off-diagonal items per core
MARGIN = 15.0             # fp8 min-candidate margin (abs, scaled by var(E))

_CACHED = {}

# ---------------------------------------------------------------------------
# schedule shared by program builder and host post-processing
# ---------------------------------------------------------------------------


def _core_cols(c):
    """Streamed col-slab ids for core c: s=0 pairs rowA with rowB's slab,
    s=1..7 pair rowA (slab c), s=8..14 pair rowB (slab 15-c)."""
    CA = [d for d in range(c + 1, 8)] + [15 - e for e in range(c)]
    CB = [15 - d for d in range(c + 1, 8)] + [e for e in range(c)]
    return [15 - c] + CA + CB  # len 15


def _coverage_check():
    got = set()
    for c in range(NCORES):
        A, B = c, 15 - c
        pairs = [(A, A), (B, B)]
        cols = _core_cols(c)
        rows = [A] * 8 + [B] * 7
        pairs += [tuple(sorted((r, cs))) for r, cs in zip(rows, cols)]
        for p in pairs:
            assert p not in got, p
            got.add(p)
    want = {(i, j) for i in range(NSLAB) for j in range(i, NSLAB)}
    assert got == want, (len(got), len(want))


_coverage_check()


def _routing():
    """Static A/B routing of fp8 pair-tile min-reductions + minb column map.

    Returns (routes, cols) where routes[(s, m)] in 'AB' and cols is the
    ordered minb column schedule: dicts describing what each column holds.
    """
    # model costs (ns)
    DVE_DIRECT = 1190.0       # fp32 TensorReduce of [128,1024] PSUM
    DVE_TT = 654.0            # bf16 TT min of [128,1024] SBUF
    DVE_CP = 325.0            # bf16 copy (first tile of a run)
    DVE_RUNRED = 1127.0       # end-of-item reduce of run tile
    ACT_CP = 925.0            # ACT copy [128,1024] PSUM->SBUF bf16
    dve = 2 * (4 * 1127.0 + 4 * 594.0)   # diag reduces (forced DVE)
    act = 16 * 190.0                      # diag block extraction copies
    cols = []
    for r in range(2):  # diag items
        for m in range(8):
            cols.append(dict(kind="diag", r=r, m=m, off=0 if m < 4 else 512))
    routes = {}
    for s in range(NF8):
        r = 0 if s < 8 else 1
        b_ms = []
        first_b = True
        for m in range(8):
            cost_b_dve = DVE_CP if first_b else DVE_TT
            # option A: dve += DVE_DIRECT ; option B: act += ACT_CP, dve += tt
            mx_a = max(dve + DVE_DIRECT, act)
            mx_b = max(dve + cost_b_dve, act + ACT_CP)
            if mx_a <= mx_b:
                routes[(s, m)] = "A"
                dve += DVE_DIRECT
                cols.append(dict(kind="f8A", s=s, r=r, m=m))
            else:
                routes[(s, m)] = "B"
                dve += cost_b_dve
                act += ACT_CP
                b_ms.append(m)
                first_b = False
        if b_ms:
            dve += DVE_RUNRED
            cols.append(dict(kind="f8B", s=s, r=r, ms=tuple(b_ms)))
    return routes, cols


_ROUTES, _COLS = _routing()
NCOLS = len(_COLS)


# ---------------------------------------------------------------------------
# device program
# ---------------------------------------------------------------------------


def _build_program():
    import concourse.bacc as bacc
    import concourse.mybir as mybir
    from concourse.tile import TileContext

    f32 = mybir.dt.float32
    bf16 = mybir.dt.bfloat16
    f8 = mybir.dt.float8e4
    DR = mybir.MatmulPerfMode.DoubleRow
    MIN = mybir.AluOpType.min
    AX = mybir.AxisListType.X

    nc = bacc.Bacc(target_bir_lowering=False)
    rows_bf = nc.declare_dram_parameter("rows_bf", [2, KC, 128, SLAB], bf16, isOutput=False)
    rows_f8 = nc.declare_dram_parameter("rows_f8", [2, KC, 128, SLAB], f8, isOutput=False)
    cols_f8 = nc.declare_dram_parameter("cols_f8", [NF8, KC, 128, SLAB], f8, isOutput=False)
    diag_out = nc.declare_dram_parameter("diag_out", [16, G, G], f32, isOutput=True)
    minb = nc.declare_dram_parameter("minb", [128, NCOLS], f32, isOutput=True)

    with TileContext(nc) as tc:
        with (
            tc.tile_pool(name="res", bufs=1) as res,
            tc.tile_pool(name="stream", bufs=3) as stream,
            tc.tile_pool(name="cps", bufs=4) as cps,
            tc.tile_pool(name="runs", bufs=2) as runs,
            tc.tile_pool(name="small", bufs=3) as small,
            tc.tile_pool(name="ps", bufs=4, space="PSUM") as ps,
        ):
            rbf = res.tile([128, 2, KC, SLAB], bf16, tag="rbf")
            r8 = res.tile([128, 2, KC, SLAB], f8, tag="r8")
            for r in range(2):
                nc.sync.dma_start(out=rbf[:, r], in_=rows_bf[r].rearrange("k p m -> p k m"))
                nc.sync.dma_start(out=r8[:, r], in_=rows_f8[r].rearrange("k p m -> p k m"))
            minbuf = res.tile([128, NCOLS], f32, tag="minbuf")

            colidx = 0

            # --- diagonal slab blocks, bf16, also emit the gathered graph blocks
            for r in range(2):
                for m in range(8):
                    pt = ps.tile([128, 1024], f32, tag="pt")
                    nlist = (0, 1) if m < 4 else (1,)
                    for n in nlist:
                        for k in range(KC):
                            nc.tensor.matmul(
                                pt[:, n * 512 : (n + 1) * 512],
                                rbf[:, r, k, m * 128 : (m + 1) * 128],
                                rbf[:, r, k, n * 512 : (n + 1) * 512],
                                start=(k == 0),
                                stop=(k == KC - 1),
                            )
                    dcp = small.tile([128, G], f32, tag="dcp")
                    nc.scalar.copy(dcp[:], pt[:, m * 128 : (m + 1) * 128])
                    nc.sync.dma_start(out=diag_out[r * 8 + m], in_=dcp[:])
                    off = 0 if m < 4 else 512
                    nc.vector.tensor_reduce(
                        minbuf[:, colidx : colidx + 1], pt[:, off:1024], AX, MIN
                    )
                    colidx += 1

            # --- fp8 DoubleRow off-diagonal sweep
            for s in range(NF8):
                r = 0 if s < 8 else 1
                ct = stream.tile([128, KC, SLAB], f8, tag="ct")
                nc.sync.dma_start(out=ct[:], in_=cols_f8[s].rearrange("k p m -> p k m"))
                runt = None
                for m in range(8):
                    pt = ps.tile([128, 1024], f32, tag="pt")
                    for n in range(2):
                        for kk in range(2):
                            nc.tensor.matmul(
                                pt[:, n * 512 : (n + 1) * 512],
                                r8[:, r, 2 * kk : 2 * kk + 2, m * 128 : (m + 1) * 128],
                                ct[:, 2 * kk : 2 * kk + 2, n * 512 : (n + 1) * 512],
                                start=(kk == 0),
                                stop=(kk == 1),
                                perf_mode=DR,
                            )
                    if _ROUTES[(s, m)] == "A":
                        nc.vector.tensor_reduce(
                            minbuf[:, colidx : colidx + 1], pt[:], AX, MIN
                        )
                        colidx += 1
                    else:
                        cb = cps.tile([128, 1024], bf16, tag="cb")
                        nc.scalar.copy(cb[:], pt[:])
                        if runt is None:
                            runt = runs.tile([128, 1024], bf16, tag="runt")
                            nc.vector.tensor_copy(runt[:], cb[:])
                        else:
                            nc.vector.tensor_tensor(runt[:], runt[:], cb[:], MIN)
                if runt is not None:
                    nc.vector.tensor_reduce(
                        minbuf[:, colidx : colidx + 1], runt[:], AX, MIN
                    )
                    colidx += 1

            assert colidx == NCOLS, (colidx, NCOLS)
            nc.sync.dma_start(out=minb[:], in_=minbuf[:])

    nc.finalize()
    return nc


# ---------------------------------------------------------------------------
# host side
# ---------------------------------------------------------------------------


def _pack_inputs(emb):
    """Per-core in_maps for the SPMD program."""
    eT = np.ascontiguousarray(emb.T)                    # [512, N] fp32
    eT4_bf = eT.astype(ml_dtypes.bfloat16).reshape(KC, 128, N)
    eT4_f8 = eT.astype(ml_dtypes.float8_e4m3).reshape(KC, 128, N)

    def slab(a, s):
        return a[:, :, s * SLAB : (s + 1) * SLAB]

    in_maps = []
    for c in range(NCORES):
        A, B = c, 15 - c
        rows_bf = np.stack([slab(eT4_bf, A), slab(eT4_bf, B)])
        rows_f8 = np.stack([slab(eT4_f8, A), slab(eT4_f8, B)])
        colslabs = _core_cols(c)
        cols = np.stack([slab(eT4_f8, cs) for cs in colslabs])
        in_maps.append(
            {
                "rows_bf": np.ascontiguousarray(rows_bf),
                "rows_f8": np.ascontiguousarray(rows_f8),
                "cols_f8": np.ascontiguousarray(cols),
            }
        )
    return in_maps


def _refine_min(emb, minb_all):
    """Exact global min: bf16-exact diag mins + host-refined fp8 candidates."""
    emb = np.asarray(emb, dtype=np.float32)
    m_diag = np.inf
    m8_min = np.inf
    for c in range(NCORES):
        for ci, col in enumerate(_COLS):
            v = float(minb_all[c][:, ci].min())
            if col["kind"] == "diag":
                m_diag = min(m_diag, v)
            else:
                m8_min = min(m8_min, v)

    margin = MARGIN * max(1.0, float(emb.std()) ** 2)
    thresh = m8_min + margin

    best = np.inf
    for c in range(NCORES):
        A, B = c, 15 - c
        colslabs = _core_cols(c)
        for ci, col in enumerate(_COLS):
            if col["kind"] == "diag":
                continue
            pcol = minb_all[c][:, ci]
            ps = np.nonzero(pcol <= thresh)[0]
            if ps.size == 0:
                continue
            row_slab = A if col["r"] == 0 else B
            col_slab = colslabs[col["s"]]
            ms = (col["m"],) if col["kind"] == "f8A" else col["ms"]
            rows = np.concatenate(
                [row_slab * SLAB + m * 128 + ps for m in ms]
            )
            cseg = emb[col_slab * SLAB : (col_slab + 1) * SLAB]
            sub = emb[rows] @ cseg.T
            best = min(best, float(sub.min()))

    return min(m_diag, best)


def kernel(embeddings, row_idx, col_idx):
    from concourse.bass_utils import run_bass_kernel_spmd

    emb = np.asarray(embeddings, dtype=np.float32)
    row_idx = np.asarray(row_idx)
    col_idx = np.asarray(col_idx)

    if "nc" not in _CACHED:
        _CACHED["nc"] = _build_program()
    nc = _CACHED["nc"]

    in_maps = _pack_inputs(emb)
    res = run_bass_kernel_spmd(nc, in_maps, list(range(NCORES)))

    # exact max via Cauchy-Schwarz: on the diagonal
    M = float(np.square(emb.astype(np.float64)).sum(axis=1).max())
    m = _refine_min(emb, [r["minb"] for r in res.results])

    blocks = np.empty((N_GRAPHS, G, G), np.float32)
    for c in range(NCORES):
        A, B = c, 15 - c
        do = res.results[c]["diag_out"]
        for i in range(8):
            blocks[A * 8 + i] = do[i]
            blocks[B * 8 + i] = do[8 + i]

    norm = (blocks - np.float32(m)) / np.float32(M - m + EPS)

    g = row_idx // G
    inblock = (col_idx // G) == g
    out = np.empty(row_idx.shape[0], np.float32)
    out[inblock] = norm[g[inblock], row_idx[inblock] % G, col_idx[inblock] % G]
    if not inblock.all():
        # fallback for non-block-diagonal gather indices: exact host dot
        bad = ~inblock
        s = np.einsum(
            "nd,nd->n", emb[row_idx[bad]], emb[col_idx[bad]], dtype=np.float64
        ).astype(np.float32)
        out[bad] = (s - np.float32(m)) / np.float32(M - m + EPS)
    return out
